# revision 18
# baseline (speedup 1.0000x reference)
"""Trainium2 Bass kernel for nn_ExtractorMLP (GNN edge cosine-similarity).

Math:  out[e] = cos_sim(mlp(emb[col[e]]), mlp(emb[row[e]]))
where  mlp(x) = elu(x @ W1.T + b1) @ W2.T + b2   (b1 = b2 = 0 here)

mlp is row-wise so mlp(emb[idx]) == mlp(emb)[idx]: compute the normalized
MLP table t[v] = g[v]/max(||g[v]||,eps) once per core (phase 1), then per
edge out[e] = dot(t[col[e]], t[row[e]]) (phase 2).

Phase 2: only the ROW side uses the GPSIMD dma_gather (its Q7
descriptor-generation ucode at ~8ns/row, serialized on the Pool engine,
was 84% of the original runtime).  The col side is recovered on the
otherwise-idle tensor engine:

  * ALL edges are sorted globally by (row_half, col) and dealt round-robin
    to the 8 cores, so every core's chunk c covers the same global window
    of the sorted col order -> identical chunk block-structure across
    cores (the SPMD program is shared) with spans of ~2 blocks.
  * col side: one-hot expansion.  onehot_j[v,e] = (colRel[e] == v + 128j)
    on DVE (iota constants from host, colRel broadcast-replicated across
    partitions by the DMA), then psum_f1[f,e] += block(b0+j) @ onehot_j
    on the tensor engine -> f1T [feat, edge] in PSUM.
  * row side: SBUF-source transposed dma_gather (int16 indices; table
    halves at 32768 nodes; the (row_half, col) sort gives each bucket one
    half) -> f2T [feat, edge] in SBUF.
  * dot: DVE multiply, then contraction over features with a sliding
    one-hot matmul packing supertile s's 512 dots into partition s%128 of
    PSUM output group s//128.

ELU identity used on device:  elu(x) = max(exp(-relu(-x)) - 1, x)
"""

import math

import numpy as np
import ml_dtypes

BF16 = ml_dtypes.bfloat16

H = 128          # feature dim
P = 128          # partitions
ST = 512         # edges per supertile (psum/output granularity)
CH = 128         # edges per col-expansion chunk
GT = 8192        # edges per dma_gather instruction
CRT = 4096       # edges per colRel broadcast load
NBUCKET = 4      # row buckets (table quarters; finer -> earlier gather start)
NCORES = 8
NSWQ = 1         # SWDGE queues: >1 corrupts concurrent gathers (racy ucode)
ST_W = 512       # phase-1 supertile width (nodes)
MAXSPAN = 6      # iota pieces provisioned (round-robin keeps spans ~2-3)

_PROG_CACHE: dict = {}
LAST_RESULTS = None  # test harness can inspect exec_time_ns


def _build_program(n_pad, qsz, nck, chunk_meta, num_devices=NCORES):
    """Build the (shared, SPMD) bass program.

    qsz: table quarter size (nodes); bucket q gathers from
         table[:, q*qsz : min((q+1)*qsz, n_pad)] with local int16 indices
    nck: per-bucket supertile counts (len NBUCKET), shared across cores
    chunk_meta: per-chunk (b0, span) merged across cores.
    """
    import concourse.bacc as bacc
    import concourse.mybir as mybir
    import concourse.tile as tile
    from concourse import library_config
    from contextlib import ExitStack

    f32 = mybir.dt.float32
    bf16 = mybir.dt.bfloat16
    i16 = mybir.dt.int16
    Alu = mybir.AluOpType
    Act = mybir.ActivationFunctionType
    Ax = mybir.AxisListType

    n_st = sum(nck)                      # total supertiles
    n_groups = math.ceil(n_st / P)
    stream_cols = n_st * (ST // 16)      # wrapped row-idx columns
    n_blocks = n_pad // H

    nc = bacc.Bacc("TRN2", target_bir_lowering=False, debug=False,
                   num_devices=num_devices, num_swdge_queues=NSWQ)

    embT = nc.dram_tensor("embT", [P, n_pad], bf16, kind="ExternalInput")
    w1t_d = nc.dram_tensor("w1t", [H, H], bf16, kind="ExternalInput")
    w2t_d = nc.dram_tensor("w2t", [H, H], bf16, kind="ExternalInput")
    ridx_d = nc.dram_tensor("ridx", [P, stream_cols], i16, kind="ExternalInput")
    crel_d = nc.dram_tensor("crel", [1, n_st * ST], i16, kind="ExternalInput")
    iota_d = nc.dram_tensor("iota", [P, MAXSPAN * ST], i16, kind="ExternalInput")
    out_d = nc.dram_tensor("out", [n_groups, P, ST], f32, kind="ExternalOutput")

    with ExitStack() as ctx:
        tc = ctx.enter_context(tile.TileContext(nc))
        const = ctx.enter_context(tc.tile_pool(name="const", bufs=1))
        p1 = ctx.enter_context(tc.tile_pool(name="p1", bufs=3))
        pf2 = ctx.enter_context(tc.tile_pool(name="pf2", bufs=2))
        pcr = ctx.enter_context(tc.tile_pool(name="pcr", bufs=2))
        poh = ctx.enter_context(tc.tile_pool(name="poh", bufs=2))
        pprod = ctx.enter_context(tc.tile_pool(name="pprod", bufs=3))
        pout_sb = ctx.enter_context(tc.tile_pool(name="pout_sb", bufs=2))
        ps1 = ctx.enter_context(tc.tile_pool(name="ps1", bufs=2, space="PSUM"))
        ps2 = ctx.enter_context(tc.tile_pool(name="ps2", bufs=2, space="PSUM"))
        psf1 = ctx.enter_context(tc.tile_pool(name="psf1", bufs=2, space="PSUM"))
        pso = ctx.enter_context(tc.tile_pool(name="pso", bufs=2, space="PSUM"))

        nc.gpsimd.load_library(library_config.mlp)

        # --- constants / persistent tiles ---
        table = const.tile([P, n_pad], bf16, tag="table")
        w1t = const.tile([H, H], bf16, tag="w1t")
        w2t = const.tile([H, H], bf16, tag="w2t")
        # sliding one-hot: onehot[:, 127-p : 255-p] has ones in column p only;
        # used as lhsT so supertile p's dot-row lands in PSUM partition p.
        onehot = const.tile([P, 2 * P - 1], bf16, tag="onehot")
        ss_all = const.tile([P, n_blocks], f32, tag="ss_all")
        r_all = const.tile([P, n_blocks], f32, tag="r_all")
        ridx = const.tile([P, stream_cols], i16, tag="ridx")
        iotas = const.tile([P, MAXSPAN * ST], i16, tag="iotas")
        nc.sync.dma_start(out=w1t[:], in_=w1t_d[:])
        nc.sync.dma_start(out=w2t[:], in_=w2t_d[:])
        nc.sync.dma_start(out=ridx[:], in_=ridx_d[:])
        nc.sync.dma_start(out=iotas[:], in_=iota_d[:])
        nc.vector.memset(onehot[:], 0.0)
        nc.vector.memset(onehot[:, P - 1:P], 1.0)

        # --- phase 1: normalized MLP table, NO global barrier: per-supertile
        # norm factors so table blocks finish progressively and the row
        # gathers of early buckets can start while phase 1 is still running.
        s_all = const.tile([P, n_blocks], f32, tag="s_all")
        m_all = const.tile([P, n_blocks], f32, tag="m_all")
        n0 = 0
        sti = 0
        while n0 < n_pad:
            w = min(ST_W, n_pad - n0)
            nb = w // H
            bsl = slice(sti * (ST_W // H), sti * (ST_W // H) + nb)
            xt = p1.tile([P, ST_W], bf16, tag="xt", name="xt")[:, :w]
            nc.sync.dma_start(out=xt, in_=embT[:, n0:n0 + w])
            ph1 = ps1.tile([P, ST_W], f32, tag="ph1", name="ph1")[:, :w]
            nc.tensor.matmul(ph1, lhsT=w1t[:], rhs=xt, start=True, stop=True)
            # elu(x) = max(exp(min(x, 0)) - 1, x)
            u_t = p1.tile([P, ST_W], bf16, tag="u", name="u")[:, :w]
            nc.vector.tensor_scalar_min(u_t, ph1, 0.0)
            e_t = p1.tile([P, ST_W], bf16, tag="e", name="e")[:, :w]
            nc.scalar.activation(e_t, u_t, Act.Exp)
            h1_t = p1.tile([P, ST_W], bf16, tag="h1", name="h1")[:, :w]
            nc.vector.scalar_tensor_tensor(
                h1_t, in0=e_t, scalar=-1.0, in1=ph1,
                op0=Alu.add, op1=Alu.max)
            pg = ps2.tile([P, ST_W], f32, tag="pg", name="pg")[:, :w]
            for b in range(nb):
                nc.tensor.matmul(pg[:, b * H:(b + 1) * H],
                                 lhsT=h1_t[:, b * H:(b + 1) * H],
                                 rhs=w2t[:], start=True, stop=True)
            # per-node sumsq -> per-supertile normalization factors
            sq_t = p1.tile([P, ST_W], bf16, tag="sq", name="sq")[:, :w]
            nc.scalar.activation(sq_t, pg, Act.Square)
            nc.vector.tensor_reduce(
                out=ss_all[:, bsl],
                in_=sq_t.rearrange("p (b f) -> p b f", b=nb),
                axis=Ax.X, op=Alu.add)
            nc.scalar.activation(s_all[:, bsl], ss_all[:, bsl], Act.Sqrt)
            nc.vector.tensor_scalar_max(m_all[:, bsl], s_all[:, bsl], 1e-8)
            nc.vector.reciprocal(r_all[:, bsl], m_all[:, bsl])
            # normalized copy PSUM -> table (per-partition scale = 1/norm)
            for b in range(nb):
                blk = sti * (ST_W // H) + b
                nc.scalar.activation(
                    table[:, n0 + b * H:n0 + (b + 1) * H],
                    pg[:, b * H:(b + 1) * H],
                    Act.Copy, scale=r_all[:, blk:blk + 1])
            n0 += w
            sti += 1

        # --- phase 2 ---
        nq = (n_pad + qsz - 1) // qsz
        quarters = tuple(table[:, q * qsz:min((q + 1) * qsz, n_pad)]
                         for q in range(nq))

        pout = None
        for hb in range(NBUCKET):
            st_base = sum(nck[:hb])
            bucket_edges = nck[hb] * ST
            t0 = 0          # edge offset within bucket
            while t0 < bucket_edges:
                tsz = min(GT, bucket_edges - t0)
                e0 = st_base * ST + t0        # global edge offset
                f2t = pf2.tile([P, GT], bf16, tag="f2", name="f2")
                f2g = f2t[:, :tsz].rearrange("p (a t) -> p a t", a=1)
                nc.gpsimd.dma_gather(
                    f2g, quarters[hb], ridx[:, e0 // 16:(e0 + tsz) // 16],
                    tsz, tsz, H,
                    transpose=True, sbuf_tokens_per_rank=P,
                    sbuf_free_dim_per_rank=256, single_packet=False,
                    queue_num=0)

                for q0 in range(0, tsz, CRT):
                    qsz = min(CRT, tsz - q0)
                    crt = pcr.tile([P, CRT], i16, tag="crt", name="crt")
                    # broadcast-replicate colRel across partitions in the DMA
                    nc.sync.dma_start(
                        out=crt[:, :qsz],
                        in_=crel_d[:, e0 + q0:e0 + q0 + qsz]
                            .to_broadcast([P, qsz]))

                    for s in range(qsz // ST):
                        stg = st_base + (t0 + q0) // ST + s
                        sl = slice(q0 + s * ST, q0 + (s + 1) * ST)
                        cl = slice(s * ST, (s + 1) * ST)
                        nj = max(chunk_meta[stg * (ST // CH) + ci][1]
                                 for ci in range(ST // CH))
                        # one batched compare builds all nj one-hot planes
                        oh = poh.tile([P, MAXSPAN * ST], bf16, tag="oh",
                                      name="oh")[:, :nj * ST]
                        nc.vector.tensor_tensor(
                            out=oh,
                            in0=crt[:, cl]
                                .rearrange("p (a t) -> p a t", a=1)
                                .to_broadcast([P, nj, ST]),
                            in1=iotas[:, :nj * ST]
                                .rearrange("p (j t) -> p j t", j=nj),
                            op=Alu.is_equal)
                        # col-side expansion into PSUM f1T [feat, edge]
                        f1p = psf1.tile([P, ST], f32, tag="f1p", name="f1p")
                        for ci in range(ST // CH):
                            b0, span = chunk_meta[stg * (ST // CH) + ci]
                            csl = slice(ci * CH, (ci + 1) * CH)
                            for j in range(span):
                                blk = b0 + j
                                nc.tensor.matmul(
                                    f1p[:, csl],
                                    lhsT=table[:, blk * H:(blk + 1) * H],
                                    rhs=oh[:, j * ST:(j + 1) * ST][:, csl],
                                    start=(j == 0), stop=(j == span - 1))
                        # dot: multiply, then pack dots via sliding onehot
                        prod = pprod.tile([P, ST], bf16, tag="prod",
                                          name="prod")
                        nc.vector.tensor_tensor(
                            out=prod[:], in0=f1p[:], in1=f2t[:, sl],
                            op=Alu.mult)
                        g, p = divmod(stg, P)
                        if p == 0:
                            pout = pso.tile([P, ST], f32, tag="pout")
                        last = stg == n_st - 1
                        nc.tensor.matmul(
                            pout[:],
                            lhsT=onehot[:, P - 1 - p:2 * P - 1 - p],
                            rhs=prod[:], start=(p == 0),
                            stop=(p == P - 1 or last))
                        if p == P - 1 or last:
                            rows = p + 1
                            ost = pout_sb.tile([P, ST], f32, tag="ost",
                                               name="ost")[:rows]
                            nc.vector.tensor_copy(out=ost, in_=pout[:rows])
                            nc.sync.dma_start(out=out_d[g, :rows], in_=ost)
                t0 += tsz

    nc.compile()
    return nc


def _wrap_idx(idx):
    """[S*16] int16 -> [128, S] wrapped layout (16 partitions, replicated 8x)."""
    w = idx.reshape(-1, 16).T.astype(np.int16)
    return np.tile(w, (8, 1))


def _ensure_ntff_hook():
    """Provide antenv.axon_hooks if the image lacks it (trace support only)."""
    import sys
    import types
    try:
        import antenv.axon_hooks  # noqa: F401
        return
    except ImportError:
        pass
    try:
        import antenv
        from trn_agent_boot.trn_boot import _ntff_profile_via_ctypes
        mod = types.ModuleType("antenv.axon_hooks")
        mod._hook = _ntff_profile_via_ctypes("/opt/axon/libaxon_pjrt.so")
        mod.get_axon_ntff_profile_hook = lambda: mod._hook
        mod.set_axon_ntff_profile_hook = lambda h: setattr(mod, "_hook", h)
        sys.modules["antenv.axon_hooks"] = mod
        antenv.axon_hooks = mod
    except Exception:
        pass


def kernel(emb, edge_index, W1, b1, W2, b2):
    global LAST_RESULTS
    from concourse.bass_utils import run_bass_kernel_spmd
    _ensure_ntff_hook()

    emb = np.asarray(emb, dtype=np.float32)
    W1 = np.asarray(W1, dtype=np.float32)
    W2 = np.asarray(W2, dtype=np.float32)
    b1 = np.asarray(b1, dtype=np.float32)
    b2 = np.asarray(b2, dtype=np.float32)
    assert np.abs(b1).max() == 0 and np.abs(b2).max() == 0, \
        "nonzero biases not implemented"
    col = np.asarray(edge_index[0]).astype(np.int64)
    row = np.asarray(edge_index[1]).astype(np.int64)

    n, h = emb.shape
    assert h == H
    E = col.shape[0]
    n_pad = ((n + P - 1) // P) * P
    qsz = math.ceil(n_pad / NBUCKET / P) * P
    n_blocks = n_pad // H

    # ---- global (row_bucket, col) sort, round-robin deal to cores ----
    qb = np.minimum(row // qsz, NBUCKET - 1).astype(np.int8)
    order = np.lexsort((col, qb))        # edge ids, bucket-0 first
    bnds = np.searchsorted(qb[order], np.arange(NBUCKET + 1))
    buckets = tuple(order[bnds[b]:bnds[b + 1]] for b in range(NBUCKET))

    percore_c = [[None] * NBUCKET for _ in range(NCORES)]
    percore_r = [[None] * NBUCKET for _ in range(NCORES)]
    nck = []
    for b in range(NBUCKET):
        ids = buckets[b]
        cnt_max = len(ids[0::NCORES]) if len(ids) else 0
        nst = math.ceil(cnt_max / ST) if cnt_max else 0
        nck.append(nst)
        tgt = nst * ST
        for ci in range(NCORES):
            sel = ids[ci::NCORES]
            cs = col[sel]
            rs = row[sel] - qsz * b
            pad = tgt - len(cs)
            fillc = cs[-1] if len(cs) else 0
            percore_c[ci][b] = np.concatenate(
                [cs, np.full(pad, fillc, dtype=cs.dtype)])
            percore_r[ci][b] = np.concatenate(
                [rs, np.zeros(pad, dtype=rs.dtype)])
    nck = tuple(nck)
    n_st = sum(nck)
    n_chunks = n_st * (ST // CH)
    n_groups = math.ceil(n_st / P)

    # merged per-chunk block structure
    b0s = np.empty((NCORES, n_chunks), dtype=np.int64)
    his = np.empty((NCORES, n_chunks), dtype=np.int64)
    for ci in range(NCORES):
        cp = np.concatenate(percore_c[ci])
        cc = cp.reshape(n_chunks, CH)
        b0s[ci] = cc[:, 0] >> 7
        his[ci] = cc[:, -1] >> 7
    b0m = b0s.min(axis=0)
    him = his.max(axis=0)
    spans = him - b0m + 1
    assert spans.max() <= MAXSPAN, f"merged span {spans.max()} > {MAXSPAN}"
    chunk_meta = tuple(zip(b0m.tolist(), spans.tolist()))

    key = (n_pad, qsz, nck, chunk_meta)
    if key not in _PROG_CACHE:
        _PROG_CACHE[key] = _build_program(n_pad, qsz, nck, chunk_meta)
    nc = _PROG_CACHE[key]

    # ---- inputs ----
    embT = np.zeros((P, n_pad), dtype=BF16)
    embT[:, :n] = emb.T.astype(BF16)
    w1t = W1.T.astype(BF16)
    w2t = W2.T.astype(BF16)
    iota = np.empty((P, MAXSPAN * ST), dtype=np.int16)
    base = np.arange(P, dtype=np.int16)[:, None]
    for j in range(MAXSPAN):
        iota[:, j * ST:(j + 1) * ST] = base + j * CH

    in_maps = []
    for ci in range(NCORES):
        cp = np.concatenate(percore_c[ci])
        rp = np.concatenate(percore_r[ci])
        crel = (cp.reshape(n_chunks, CH)
                - (b0m[:, None] << 7)).astype(np.int16).reshape(1, -1)
        assert crel.min() >= 0 and crel.max() < MAXSPAN * CH
        in_maps.append({
            "embT": embT, "w1t": w1t, "w2t": w2t,
            "ridx": _wrap_idx(rp.astype(np.int16)),
            "crel": crel, "iota": iota,
        })

    res = run_bass_kernel_spmd(nc, in_maps, core_ids=list(range(NCORES)))
    LAST_RESULTS = res

    # ---- reassemble ----
    out = np.empty(E, dtype=np.float32)
    streams = [res.results[ci]["out"].reshape(-1) for ci in range(NCORES)]
    for b in range(NBUCKET):
        ids = buckets[b]
        off = sum(nck[:b]) * ST
        for ci in range(NCORES):
            sel = ids[ci::NCORES]
            out[sel] = streams[ci][off:off + len(sel)]
    return out


# revision 19
# speedup vs baseline: 1.1413x; 1.1413x over previous
"""Trainium2 Bass kernel for nn_ExtractorMLP (GNN edge cosine-similarity).

Math:  out[e] = cos_sim(mlp(emb[col[e]]), mlp(emb[row[e]]))
where  mlp(x) = elu(x @ W1.T + b1) @ W2.T + b2   (b1 = b2 = 0 here)

mlp is row-wise so mlp(emb[idx]) == mlp(emb)[idx]: compute the normalized
MLP table t[v] = g[v]/max(||g[v]||,eps) once per core (phase 1), then per
edge out[e] = dot(t[col[e]], t[row[e]]) (phase 2).

Phase 2: only the ROW side uses the GPSIMD dma_gather (its Q7
descriptor-generation ucode at ~8ns/row, serialized on the Pool engine,
was 84% of the original runtime).  The col side is recovered on the
otherwise-idle tensor engine:

  * ALL edges are sorted globally by (row_half, col) and dealt round-robin
    to the 8 cores, so every core's chunk c covers the same global window
    of the sorted col order -> identical chunk block-structure across
    cores (the SPMD program is shared) with spans of ~2 blocks.
  * col side: one-hot expansion.  onehot_j[v,e] = (colRel[e] == v + 128j)
    on DVE (iota constants from host, colRel broadcast-replicated across
    partitions by the DMA), then psum_f1[f,e] += block(b0+j) @ onehot_j
    on the tensor engine -> f1T [feat, edge] in PSUM.
  * row side: SBUF-source transposed dma_gather (int16 indices; table
    halves at 32768 nodes; the (row_half, col) sort gives each bucket one
    half) -> f2T [feat, edge] in SBUF.
  * dot: DVE multiply, then contraction over features with a sliding
    one-hot matmul packing supertile s's 512 dots into partition s%128 of
    PSUM output group s//128.

ELU identity used on device:  elu(x) = max(exp(-relu(-x)) - 1, x)
"""

import math

import numpy as np
import ml_dtypes

BF16 = ml_dtypes.bfloat16

H = 128          # feature dim
P = 128          # partitions
ST = 512         # edges per supertile (psum/output granularity)
CH = 128         # edges per col-expansion chunk
GT = 4096        # edges per dma_gather instruction
CRT = 4096       # edges per colRel broadcast load
NBUCKET = 4      # row buckets (table quarters; finer -> earlier gather start)
NCORES = 8
NSWQ = 1         # SWDGE queues: >1 corrupts concurrent gathers (racy ucode)
ST_W = 512       # phase-1 supertile width (nodes)
MAXSPAN = 6      # iota pieces provisioned (round-robin keeps spans ~2-3)

_PROG_CACHE: dict = {}
LAST_RESULTS = None  # test harness can inspect exec_time_ns


def _build_program(n_pad, qsz, nck, chunk_meta, num_devices=NCORES):
    """Build the (shared, SPMD) bass program.

    qsz: table quarter size (nodes); bucket q gathers from
         table[:, q*qsz : min((q+1)*qsz, n_pad)] with local int16 indices
    nck: per-bucket supertile counts (len NBUCKET), shared across cores
    chunk_meta: per-chunk (b0, span) merged across cores.
    """
    import concourse.bacc as bacc
    import concourse.mybir as mybir
    import concourse.tile as tile
    from concourse import library_config
    from contextlib import ExitStack

    f32 = mybir.dt.float32
    bf16 = mybir.dt.bfloat16
    i16 = mybir.dt.int16
    f16 = mybir.dt.float16
    Alu = mybir.AluOpType
    Act = mybir.ActivationFunctionType
    Ax = mybir.AxisListType

    n_st = sum(nck)                      # total supertiles
    n_groups = math.ceil(n_st / P)
    stream_cols = n_st * (ST // 16)      # wrapped row-idx columns
    n_blocks = n_pad // H

    nc = bacc.Bacc("TRN2", target_bir_lowering=False, debug=False,
                   num_devices=num_devices, num_swdge_queues=NSWQ)

    embT = nc.dram_tensor("embT", [P, n_pad], bf16, kind="ExternalInput")
    w1t_d = nc.dram_tensor("w1t", [H, H], bf16, kind="ExternalInput")
    w2t_d = nc.dram_tensor("w2t", [H, H], bf16, kind="ExternalInput")
    ridx_d = nc.dram_tensor("ridx", [P, stream_cols], i16, kind="ExternalInput")
    crel_d = nc.dram_tensor("crel", [1, n_st * ST], f16, kind="ExternalInput")
    iota_d = nc.dram_tensor("iota", [P, MAXSPAN * ST], f16, kind="ExternalInput")
    out_d = nc.dram_tensor("out", [n_groups, P, ST], f32, kind="ExternalOutput")

    with ExitStack() as ctx:
        tc = ctx.enter_context(tile.TileContext(nc))
        const = ctx.enter_context(tc.tile_pool(name="const", bufs=1))
        p1 = ctx.enter_context(tc.tile_pool(name="p1", bufs=3))
        pf2 = ctx.enter_context(tc.tile_pool(name="pf2", bufs=3))
        pcr = ctx.enter_context(tc.tile_pool(name="pcr", bufs=2))
        poh = ctx.enter_context(tc.tile_pool(name="poh", bufs=2))
        pprod = ctx.enter_context(tc.tile_pool(name="pprod", bufs=3))
        pout_sb = ctx.enter_context(tc.tile_pool(name="pout_sb", bufs=2))
        ps1 = ctx.enter_context(tc.tile_pool(name="ps1", bufs=2, space="PSUM"))
        ps2 = ctx.enter_context(tc.tile_pool(name="ps2", bufs=2, space="PSUM"))
        psf1 = ctx.enter_context(tc.tile_pool(name="psf1", bufs=2, space="PSUM"))
        pso = ctx.enter_context(tc.tile_pool(name="pso", bufs=2, space="PSUM"))

        nc.gpsimd.load_library(library_config.mlp)

        # --- constants / persistent tiles ---
        table = const.tile([P, n_pad], bf16, tag="table")
        w1t = const.tile([H, H], bf16, tag="w1t")
        w2t = const.tile([H, H], bf16, tag="w2t")
        # sliding one-hot: onehot[:, 127-p : 255-p] has ones in column p only;
        # used as lhsT so supertile p's dot-row lands in PSUM partition p.
        onehot = const.tile([P, 2 * P - 1], bf16, tag="onehot")
        ss_all = const.tile([P, n_blocks], f32, tag="ss_all")
        r_all = const.tile([P, n_blocks], f32, tag="r_all")
        ridx = const.tile([P, stream_cols], i16, tag="ridx")
        iotas = const.tile([P, MAXSPAN * ST], f16, tag="iotas")
        nc.sync.dma_start(out=w1t[:], in_=w1t_d[:])
        nc.sync.dma_start(out=w2t[:], in_=w2t_d[:])
        nc.sync.dma_start(out=ridx[:], in_=ridx_d[:])
        nc.sync.dma_start(out=iotas[:], in_=iota_d[:])
        nc.vector.memset(onehot[:], 0.0)
        nc.vector.memset(onehot[:, P - 1:P], 1.0)

        # --- phase 1: normalized MLP table, NO global barrier: per-supertile
        # norm factors so table blocks finish progressively and the row
        # gathers of early buckets can start while phase 1 is still running.
        s_all = const.tile([P, n_blocks], f32, tag="s_all")
        m_all = const.tile([P, n_blocks], f32, tag="m_all")
        n0 = 0
        sti = 0
        while n0 < n_pad:
            w = min(ST_W, n_pad - n0)
            nb = w // H
            bsl = slice(sti * (ST_W // H), sti * (ST_W // H) + nb)
            xt = p1.tile([P, ST_W], bf16, tag="xt", name="xt")[:, :w]
            nc.sync.dma_start(out=xt, in_=embT[:, n0:n0 + w])
            ph1 = ps1.tile([P, ST_W], f32, tag="ph1", name="ph1")[:, :w]
            nc.tensor.matmul(ph1, lhsT=w1t[:], rhs=xt, start=True, stop=True)
            # elu(x) = max(exp(min(x, 0)) - 1, x)
            u_t = p1.tile([P, ST_W], bf16, tag="u", name="u")[:, :w]
            nc.scalar.activation(u_t, ph1, Act.Relu, scale=-1.0)
            e_t = p1.tile([P, ST_W], bf16, tag="e", name="e")[:, :w]
            nc.scalar.activation(e_t, u_t, Act.Exp, scale=-1.0)
            h1_t = p1.tile([P, ST_W], bf16, tag="h1", name="h1")[:, :w]
            nc.vector.scalar_tensor_tensor(
                h1_t, in0=e_t, scalar=-1.0, in1=ph1,
                op0=Alu.add, op1=Alu.max)
            pg = ps2.tile([P, ST_W], f32, tag="pg", name="pg")[:, :w]
            for b in range(nb):
                nc.tensor.matmul(pg[:, b * H:(b + 1) * H],
                                 lhsT=h1_t[:, b * H:(b + 1) * H],
                                 rhs=w2t[:], start=True, stop=True)
            # per-node sumsq -> per-supertile normalization factors
            sq_t = p1.tile([P, ST_W], bf16, tag="sq", name="sq")[:, :w]
            nc.scalar.activation(sq_t, pg, Act.Square)
            nc.vector.tensor_reduce(
                out=ss_all[:, bsl],
                in_=sq_t.rearrange("p (b f) -> p b f", b=nb),
                axis=Ax.X, op=Alu.add)
            nc.scalar.activation(s_all[:, bsl], ss_all[:, bsl], Act.Sqrt)
            nc.vector.tensor_scalar_max(m_all[:, bsl], s_all[:, bsl], 1e-8)
            nc.vector.reciprocal(r_all[:, bsl], m_all[:, bsl])
            # normalized copy PSUM -> table (per-partition scale = 1/norm)
            for b in range(nb):
                blk = sti * (ST_W // H) + b
                nc.scalar.activation(
                    table[:, n0 + b * H:n0 + (b + 1) * H],
                    pg[:, b * H:(b + 1) * H],
                    Act.Copy, scale=r_all[:, blk:blk + 1])
            n0 += w
            sti += 1

        # --- phase 2 ---
        nq = (n_pad + qsz - 1) // qsz
        quarters = tuple(table[:, q * qsz:min((q + 1) * qsz, n_pad)]
                         for q in range(nq))

        pout = None
        for hb in range(NBUCKET):
            st_base = sum(nck[:hb])
            bucket_edges = nck[hb] * ST
            t0 = 0          # edge offset within bucket
            while t0 < bucket_edges:
                tsz = min(GT, bucket_edges - t0)
                e0 = st_base * ST + t0        # global edge offset
                f2t = pf2.tile([P, GT], bf16, tag="f2", name="f2")
                f2g = f2t[:, :tsz].rearrange("p (a t) -> p a t", a=1)
                nc.gpsimd.dma_gather(
                    f2g, quarters[hb], ridx[:, e0 // 16:(e0 + tsz) // 16],
                    tsz, tsz, H,
                    transpose=True, sbuf_tokens_per_rank=P,
                    sbuf_free_dim_per_rank=256, single_packet=False,
                    queue_num=0)

                for q0 in range(0, tsz, CRT):
                    qsz = min(CRT, tsz - q0)
                    crt = pcr.tile([P, CRT], f16, tag="crt", name="crt")
                    # broadcast-replicate colRel across partitions in the DMA
                    nc.sync.dma_start(
                        out=crt[:, :qsz],
                        in_=crel_d[:, e0 + q0:e0 + q0 + qsz]
                            .to_broadcast([P, qsz]))

                    for s in range(qsz // ST):
                        stg = st_base + (t0 + q0) // ST + s
                        sl = slice(q0 + s * ST, q0 + (s + 1) * ST)
                        cl = slice(s * ST, (s + 1) * ST)
                        nj = max(chunk_meta[stg * (ST // CH) + ci][1]
                                 for ci in range(ST // CH))
                        # one batched compare builds all nj one-hot planes
                        oh = poh.tile([P, MAXSPAN * ST], bf16, tag="oh",
                                      name="oh")[:, :nj * ST]
                        nc.vector.tensor_tensor(
                            out=oh,
                            in0=crt[:, cl]
                                .rearrange("p (a t) -> p a t", a=1)
                                .to_broadcast([P, nj, ST]),
                            in1=iotas[:, :nj * ST]
                                .rearrange("p (j t) -> p j t", j=nj),
                            op=Alu.is_equal)
                        # col-side expansion into PSUM f1T [feat, edge]
                        f1p = psf1.tile([P, ST], f32, tag="f1p", name="f1p")
                        for ci in range(ST // CH):
                            b0, span = chunk_meta[stg * (ST // CH) + ci]
                            csl = slice(ci * CH, (ci + 1) * CH)
                            for j in range(span):
                                blk = b0 + j
                                nc.tensor.matmul(
                                    f1p[:, csl],
                                    lhsT=table[:, blk * H:(blk + 1) * H],
                                    rhs=oh[:, j * ST:(j + 1) * ST][:, csl],
                                    start=(j == 0), stop=(j == span - 1))
                        # dot: multiply, then pack dots via sliding onehot
                        prod = pprod.tile([P, ST], bf16, tag="prod",
                                          name="prod")
                        nc.vector.tensor_tensor(
                            out=prod[:], in0=f1p[:], in1=f2t[:, sl],
                            op=Alu.mult)
                        g, p = divmod(stg, P)
                        if p == 0:
                            pout = pso.tile([P, ST], f32, tag="pout")
                        last = stg == n_st - 1
                        nc.tensor.matmul(
                            pout[:],
                            lhsT=onehot[:, P - 1 - p:2 * P - 1 - p],
                            rhs=prod[:], start=(p == 0),
                            stop=(p == P - 1 or last))
                        if p == P - 1 or last:
                            rows = p + 1
                            ost = pout_sb.tile([P, ST], f32, tag="ost",
                                               name="ost")[:rows]
                            nc.vector.tensor_copy(out=ost, in_=pout[:rows])
                            nc.sync.dma_start(out=out_d[g, :rows], in_=ost)
                t0 += tsz

    nc.compile()
    return nc


def _wrap_idx(idx):
    """[S*16] int16 -> [128, S] wrapped layout (16 partitions, replicated 8x)."""
    w = idx.reshape(-1, 16).T.astype(np.int16)
    return np.tile(w, (8, 1))


def _ensure_ntff_hook():
    """Provide antenv.axon_hooks if the image lacks it (trace support only)."""
    import sys
    import types
    try:
        import antenv.axon_hooks  # noqa: F401
        return
    except ImportError:
        pass
    try:
        import antenv
        from trn_agent_boot.trn_boot import _ntff_profile_via_ctypes
        mod = types.ModuleType("antenv.axon_hooks")
        mod._hook = _ntff_profile_via_ctypes("/opt/axon/libaxon_pjrt.so")
        mod.get_axon_ntff_profile_hook = lambda: mod._hook
        mod.set_axon_ntff_profile_hook = lambda h: setattr(mod, "_hook", h)
        sys.modules["antenv.axon_hooks"] = mod
        antenv.axon_hooks = mod
    except Exception:
        pass


def kernel(emb, edge_index, W1, b1, W2, b2):
    global LAST_RESULTS
    from concourse.bass_utils import run_bass_kernel_spmd
    _ensure_ntff_hook()

    emb = np.asarray(emb, dtype=np.float32)
    W1 = np.asarray(W1, dtype=np.float32)
    W2 = np.asarray(W2, dtype=np.float32)
    b1 = np.asarray(b1, dtype=np.float32)
    b2 = np.asarray(b2, dtype=np.float32)
    assert np.abs(b1).max() == 0 and np.abs(b2).max() == 0, \
        "nonzero biases not implemented"
    col = np.asarray(edge_index[0]).astype(np.int64)
    row = np.asarray(edge_index[1]).astype(np.int64)

    n, h = emb.shape
    assert h == H
    E = col.shape[0]
    n_pad = ((n + P - 1) // P) * P
    qsz = math.ceil(n_pad / NBUCKET / P) * P
    n_blocks = n_pad // H

    # ---- global (row_bucket, col) sort, round-robin deal to cores ----
    qb = np.minimum(row // qsz, NBUCKET - 1).astype(np.int8)
    order = np.lexsort((col, qb))        # edge ids, bucket-0 first
    bnds = np.searchsorted(qb[order], np.arange(NBUCKET + 1))
    buckets = tuple(order[bnds[b]:bnds[b + 1]] for b in range(NBUCKET))

    percore_c = [[None] * NBUCKET for _ in range(NCORES)]
    percore_r = [[None] * NBUCKET for _ in range(NCORES)]
    nck = []
    for b in range(NBUCKET):
        ids = buckets[b]
        cnt_max = len(ids[0::NCORES]) if len(ids) else 0
        nst = math.ceil(cnt_max / ST) if cnt_max else 0
        nck.append(nst)
        tgt = nst * ST
        for ci in range(NCORES):
            sel = ids[ci::NCORES]
            cs = col[sel]
            rs = row[sel] - qsz * b
            pad = tgt - len(cs)
            fillc = cs[-1] if len(cs) else 0
            percore_c[ci][b] = np.concatenate(
                [cs, np.full(pad, fillc, dtype=cs.dtype)])
            percore_r[ci][b] = np.concatenate(
                [rs, np.zeros(pad, dtype=rs.dtype)])
    nck = tuple(nck)
    n_st = sum(nck)
    n_chunks = n_st * (ST // CH)
    n_groups = math.ceil(n_st / P)

    # merged per-chunk block structure
    b0s = np.empty((NCORES, n_chunks), dtype=np.int64)
    his = np.empty((NCORES, n_chunks), dtype=np.int64)
    for ci in range(NCORES):
        cp = np.concatenate(percore_c[ci])
        cc = cp.reshape(n_chunks, CH)
        b0s[ci] = cc[:, 0] >> 7
        his[ci] = cc[:, -1] >> 7
    b0m = b0s.min(axis=0)
    him = his.max(axis=0)
    spans = him - b0m + 1
    assert spans.max() <= MAXSPAN, f"merged span {spans.max()} > {MAXSPAN}"
    chunk_meta = tuple(zip(b0m.tolist(), spans.tolist()))

    key = (n_pad, qsz, nck, chunk_meta)
    if key not in _PROG_CACHE:
        _PROG_CACHE[key] = _build_program(n_pad, qsz, nck, chunk_meta)
    nc = _PROG_CACHE[key]

    # ---- inputs ----
    embT = np.zeros((P, n_pad), dtype=BF16)
    embT[:, :n] = emb.T.astype(BF16)
    w1t = W1.T.astype(BF16)
    w2t = W2.T.astype(BF16)
    iota = np.empty((P, MAXSPAN * ST), dtype=np.float16)
    base = np.arange(P, dtype=np.float16)[:, None]
    for j in range(MAXSPAN):
        iota[:, j * ST:(j + 1) * ST] = base + j * CH

    in_maps = []
    for ci in range(NCORES):
        cp = np.concatenate(percore_c[ci])
        rp = np.concatenate(percore_r[ci])
        crel = (cp.reshape(n_chunks, CH)
                - (b0m[:, None] << 7)).astype(np.float16).reshape(1, -1)
        assert crel.min() >= 0 and crel.max() < MAXSPAN * CH
        in_maps.append({
            "embT": embT, "w1t": w1t, "w2t": w2t,
            "ridx": _wrap_idx(rp.astype(np.int16)),
            "crel": crel, "iota": iota,
        })

    res = run_bass_kernel_spmd(nc, in_maps, core_ids=list(range(NCORES)))
    LAST_RESULTS = res

    # ---- reassemble ----
    out = np.empty(E, dtype=np.float32)
    streams = [res.results[ci]["out"].reshape(-1) for ci in range(NCORES)]
    for b in range(NBUCKET):
        ids = buckets[b]
        off = sum(nck[:b]) * ST
        for ci in range(NCORES):
            sel = ids[ci::NCORES]
            out[sel] = streams[ci][off:off + len(sel)]
    return out


# revision 20
# speedup vs baseline: 1.3538x; 1.1862x over previous
"""Trainium2 Bass kernel for nn_ExtractorMLP (GNN edge cosine-similarity).

Math:  out[e] = cos_sim(mlp(emb[col[e]]), mlp(emb[row[e]]))
where  mlp(x) = elu(x @ W1.T + b1) @ W2.T + b2   (b1 = b2 = 0 here)

mlp is row-wise so mlp(emb[idx]) == mlp(emb)[idx]: compute the normalized
MLP table t[v] = g[v]/max(||g[v]||,eps) once per core (phase 1), then per
edge out[e] = dot(t[col[e]], t[row[e]]) (phase 2).

Phase 2: only the ROW side uses the GPSIMD dma_gather (its Q7
descriptor-generation ucode at ~8ns/row, serialized on the Pool engine,
was 84% of the original runtime).  The col side is recovered on the
otherwise-idle tensor engine:

  * ALL edges are sorted globally by (row_half, col) and dealt round-robin
    to the 8 cores, so every core's chunk c covers the same global window
    of the sorted col order -> identical chunk block-structure across
    cores (the SPMD program is shared) with spans of ~2 blocks.
  * col side: one-hot expansion.  onehot_j[v,e] = (colRel[e] == v + 128j)
    on DVE (iota constants from host, colRel broadcast-replicated across
    partitions by the DMA), then psum_f1[f,e] += block(b0+j) @ onehot_j
    on the tensor engine -> f1T [feat, edge] in PSUM.
  * row side: SBUF-source transposed dma_gather (int16 indices; table
    halves at 32768 nodes; the (row_half, col) sort gives each bucket one
    half) -> f2T [feat, edge] in SBUF.
  * dot: DVE multiply, then contraction over features with a sliding
    one-hot matmul packing supertile s's 512 dots into partition s%128 of
    PSUM output group s//128.

ELU identity used on device:  elu(x) = max(exp(-relu(-x)) - 1, x)
"""

import math

import numpy as np
import ml_dtypes

BF16 = ml_dtypes.bfloat16

H = 128          # feature dim
P = 128          # partitions
ST = 512         # edges per supertile (psum/output granularity)
CH = 128         # edges per col-expansion chunk
GT = 4096        # edges per dma_gather instruction
CRT = 4096       # edges per colRel broadcast load
NBUCKET = 4      # row buckets (table quarters; finer -> earlier gather start)
NCORES = 8
NSWQ = 1         # SWDGE queues: >1 corrupts concurrent gathers (racy ucode)
ST_W = 512       # phase-1 supertile width (nodes)
MAXSPAN = 6      # iota pieces provisioned (round-robin keeps spans ~2-3)

_PROG_CACHE: dict = {}
LAST_RESULTS = None  # test harness can inspect exec_time_ns


def _build_program(n_pad, qsz, nck, chunk_meta, num_devices=NCORES):
    """Build the (shared, SPMD) bass program.

    qsz: table quarter size (nodes); bucket q gathers from
         table[:, q*qsz : min((q+1)*qsz, n_pad)] with local int16 indices
    nck: per-bucket supertile counts (len NBUCKET), shared across cores
    chunk_meta: per-chunk (b0, span) merged across cores.
    """
    import concourse.bacc as bacc
    import concourse.mybir as mybir
    import concourse.tile as tile
    from concourse import library_config
    from contextlib import ExitStack

    f32 = mybir.dt.float32
    bf16 = mybir.dt.bfloat16
    i16 = mybir.dt.int16
    f16 = mybir.dt.float16
    Alu = mybir.AluOpType
    Act = mybir.ActivationFunctionType
    Ax = mybir.AxisListType

    n_st = sum(nck)                      # total supertiles
    n_groups = math.ceil(n_st / P)
    stream_cols = n_st * (ST // 16)      # wrapped row-idx columns
    n_blocks = n_pad // H

    nc = bacc.Bacc("TRN2", target_bir_lowering=False, debug=False,
                   num_devices=num_devices, num_swdge_queues=NSWQ)

    embT = nc.dram_tensor("embT", [P, n_pad], bf16, kind="ExternalInput")
    w1t_d = nc.dram_tensor("w1t", [H, H], bf16, kind="ExternalInput")
    w2t_d = nc.dram_tensor("w2t", [H, H], bf16, kind="ExternalInput")
    ridx_d = nc.dram_tensor("ridx", [P, stream_cols], i16, kind="ExternalInput")
    crel_d = nc.dram_tensor("crel", [1, n_st * ST], f16, kind="ExternalInput")
    iota_d = nc.dram_tensor("iota", [P, MAXSPAN * ST], f16, kind="ExternalInput")
    out_d = nc.dram_tensor("out", [n_groups, P, ST], f32, kind="ExternalOutput")

    with ExitStack() as ctx:
        tc = ctx.enter_context(tile.TileContext(nc))
        const = ctx.enter_context(tc.tile_pool(name="const", bufs=1))
        p1 = ctx.enter_context(tc.tile_pool(name="p1", bufs=3))
        pf2 = ctx.enter_context(tc.tile_pool(name="pf2", bufs=3))
        pcr = ctx.enter_context(tc.tile_pool(name="pcr", bufs=2))
        poh = ctx.enter_context(tc.tile_pool(name="poh", bufs=3))
        pprod = ctx.enter_context(tc.tile_pool(name="pprod", bufs=3))
        pout_sb = ctx.enter_context(tc.tile_pool(name="pout_sb", bufs=2))
        ps1 = ctx.enter_context(tc.tile_pool(name="ps1", bufs=2, space="PSUM"))
        ps2 = ctx.enter_context(tc.tile_pool(name="ps2", bufs=2, space="PSUM"))
        psf1 = ctx.enter_context(tc.tile_pool(name="psf1", bufs=3, space="PSUM"))
        pso = ctx.enter_context(tc.tile_pool(name="pso", bufs=1, space="PSUM"))

        nc.gpsimd.load_library(library_config.mlp)

        # --- constants / persistent tiles ---
        table = const.tile([P, n_pad], bf16, tag="table")
        w1t = const.tile([H, H], bf16, tag="w1t")
        w2t = const.tile([H, H], bf16, tag="w2t")
        # sliding one-hot: onehot[:, 127-p : 255-p] has ones in column p only;
        # used as lhsT so supertile p's dot-row lands in PSUM partition p.
        onehot = const.tile([P, 2 * P - 1], bf16, tag="onehot")
        ss_all = const.tile([P, n_blocks], f32, tag="ss_all")
        r_all = const.tile([P, n_blocks], f32, tag="r_all")
        ridx = const.tile([P, stream_cols], i16, tag="ridx")
        iotas = const.tile([P, MAXSPAN * ST], f16, tag="iotas")
        nc.sync.dma_start(out=w1t[:], in_=w1t_d[:])
        nc.sync.dma_start(out=w2t[:], in_=w2t_d[:])
        nc.sync.dma_start(out=ridx[:], in_=ridx_d[:])
        nc.sync.dma_start(out=iotas[:], in_=iota_d[:])
        nc.vector.memset(onehot[:], 0.0)
        nc.vector.memset(onehot[:, P - 1:P], 1.0)

        # --- phase 1: normalized MLP table.  Normalization factors are
        # computed per table QUARTER (the row-gather granularity) so early
        # buckets' gathers can start while later quarters are still being
        # built.  ACT runs only Exp + (untabled) Copy in the steady loop --
        # Sqrt appears once per quarter -- to avoid per-supertile activation
        # table reloads (1.3us each).
        s_all = const.tile([P, n_blocks], f32, tag="s_all")
        m_all = const.tile([P, n_blocks], f32, tag="m_all")
        nq = (n_pad + qsz - 1) // qsz
        sti = 0
        for q in range(nq):
            q0 = q * qsz
            q1 = min((q + 1) * qsz, n_pad)
            n0 = q0
            while n0 < q1:
                w = min(ST_W, q1 - n0)
                nb = w // H
                bsl = slice(sti * (ST_W // H), sti * (ST_W // H) + nb)
                xt = p1.tile([P, ST_W], bf16, tag="xt", name="xt")[:, :w]
                nc.sync.dma_start(out=xt, in_=embT[:, n0:n0 + w])
                ph1 = ps1.tile([P, ST_W], f32, tag="ph1", name="ph1")[:, :w]
                nc.tensor.matmul(ph1, lhsT=w1t[:], rhs=xt, start=True,
                                 stop=True)
                # elu(x) = max(exp(min(x, 0)) - 1, x)
                u_t = p1.tile([P, ST_W], bf16, tag="u", name="u")[:, :w]
                nc.vector.tensor_scalar_min(u_t, ph1, 0.0)
                e_t = p1.tile([P, ST_W], bf16, tag="e", name="e")[:, :w]
                nc.scalar.activation(e_t, u_t, Act.Exp)
                h1_t = p1.tile([P, ST_W], bf16, tag="h1", name="h1")[:, :w]
                nc.vector.scalar_tensor_tensor(
                    h1_t, in0=e_t, scalar=-1.0, in1=ph1,
                    op0=Alu.add, op1=Alu.max)
                pg = ps2.tile([P, ST_W], f32, tag="pg", name="pg")[:, :w]
                for b in range(nb):
                    nc.tensor.matmul(pg[:, b * H:(b + 1) * H],
                                     lhsT=h1_t[:, b * H:(b + 1) * H],
                                     rhs=w2t[:], start=True, stop=True)
                # stage unnormalized G; accumulate per-node sumsq
                nc.scalar.activation(table[:, n0:n0 + w], pg, Act.Copy)
                sq_t = p1.tile([P, ST_W], bf16, tag="sq", name="sq")[:, :w]
                nc.vector.scalar_tensor_tensor(
                    sq_t, in0=pg, scalar=0.0, in1=table[:, n0:n0 + w],
                    op0=Alu.add, op1=Alu.mult)
                nc.vector.tensor_reduce(
                    out=ss_all[:, bsl],
                    in_=sq_t.rearrange("p (b f) -> p b f", b=nb),
                    axis=Ax.X, op=Alu.add)
                n0 += w
                sti += 1
            # quarter normalization: factors + in-place scaled copies
            qb0, qb1 = q0 // H, (q1 + H - 1) // H
            qsl = slice(qb0, qb1)
            nc.scalar.activation(s_all[:, qsl], ss_all[:, qsl], Act.Sqrt)
            nc.vector.tensor_scalar_max(m_all[:, qsl], s_all[:, qsl], 1e-8)
            nc.vector.reciprocal(r_all[:, qsl], m_all[:, qsl])
            for blk in range(qb0, qb1):
                nc.scalar.activation(
                    table[:, blk * H:(blk + 1) * H],
                    table[:, blk * H:(blk + 1) * H],
                    Act.Copy, scale=r_all[:, blk:blk + 1])

        # --- phase 2 ---
        quarters = tuple(table[:, q * qsz:min((q + 1) * qsz, n_pad)]
                         for q in range(nq))

        pout = None
        for hb in range(NBUCKET):
            st_base = sum(nck[:hb])
            bucket_edges = nck[hb] * ST
            t0 = 0          # edge offset within bucket
            while t0 < bucket_edges:
                tsz = min(GT, bucket_edges - t0)
                e0 = st_base * ST + t0        # global edge offset
                f2t = pf2.tile([P, GT], bf16, tag="f2", name="f2")
                f2g = f2t[:, :tsz].rearrange("p (a t) -> p a t", a=1)
                nc.gpsimd.dma_gather(
                    f2g, quarters[hb], ridx[:, e0 // 16:(e0 + tsz) // 16],
                    tsz, tsz, H,
                    transpose=True, sbuf_tokens_per_rank=P,
                    sbuf_free_dim_per_rank=256, single_packet=False,
                    queue_num=0)

                for q0 in range(0, tsz, CRT):
                    qsz = min(CRT, tsz - q0)
                    crt = pcr.tile([P, CRT], f16, tag="crt", name="crt")
                    # broadcast-replicate colRel across partitions in the DMA
                    nc.sync.dma_start(
                        out=crt[:, :qsz],
                        in_=crel_d[:, e0 + q0:e0 + q0 + qsz]
                            .to_broadcast([P, qsz]))

                    for s in range(qsz // ST):
                        stg = st_base + (t0 + q0) // ST + s
                        sl = slice(q0 + s * ST, q0 + (s + 1) * ST)
                        cl = slice(s * ST, (s + 1) * ST)
                        nj = max(chunk_meta[stg * (ST // CH) + ci][1]
                                 for ci in range(ST // CH))
                        # one batched compare builds all nj one-hot planes
                        oh = poh.tile([P, MAXSPAN * ST], bf16, tag="oh",
                                      name="oh")[:, :nj * ST]
                        nc.vector.tensor_tensor(
                            out=oh,
                            in0=crt[:, cl]
                                .rearrange("p (a t) -> p a t", a=1)
                                .to_broadcast([P, nj, ST]),
                            in1=iotas[:, :nj * ST]
                                .rearrange("p (j t) -> p j t", j=nj),
                            op=Alu.is_equal)
                        # col-side expansion into PSUM f1T [feat, edge]
                        f1p = psf1.tile([P, ST], f32, tag="f1p", name="f1p")
                        for ci in range(ST // CH):
                            b0, span = chunk_meta[stg * (ST // CH) + ci]
                            csl = slice(ci * CH, (ci + 1) * CH)
                            for j in range(span):
                                blk = b0 + j
                                nc.tensor.matmul(
                                    f1p[:, csl],
                                    lhsT=table[:, blk * H:(blk + 1) * H],
                                    rhs=oh[:, j * ST:(j + 1) * ST][:, csl],
                                    start=(j == 0), stop=(j == span - 1))
                        # dot: multiply, then pack dots via sliding onehot
                        prod = pprod.tile([P, ST], bf16, tag="prod",
                                          name="prod")
                        nc.vector.tensor_tensor(
                            out=prod[:], in0=f1p[:], in1=f2t[:, sl],
                            op=Alu.mult)
                        g, p = divmod(stg, P)
                        if p == 0:
                            pout = pso.tile([P, ST], f32, tag="pout")
                        last = stg == n_st - 1
                        nc.tensor.matmul(
                            pout[:],
                            lhsT=onehot[:, P - 1 - p:2 * P - 1 - p],
                            rhs=prod[:], start=(p == 0),
                            stop=(p == P - 1 or last))
                        if p == P - 1 or last:
                            rows = p + 1
                            ost = pout_sb.tile([P, ST], f32, tag="ost",
                                               name="ost")[:rows]
                            nc.vector.tensor_copy(out=ost, in_=pout[:rows])
                            nc.sync.dma_start(out=out_d[g, :rows], in_=ost)
                t0 += tsz

    nc.compile()
    return nc


def _wrap_idx(idx):
    """[S*16] int16 -> [128, S] wrapped layout (16 partitions, replicated 8x)."""
    w = idx.reshape(-1, 16).T.astype(np.int16)
    return np.tile(w, (8, 1))


def _ensure_ntff_hook():
    """Provide antenv.axon_hooks if the image lacks it (trace support only)."""
    import sys
    import types
    try:
        import antenv.axon_hooks  # noqa: F401
        return
    except ImportError:
        pass
    try:
        import antenv
        from trn_agent_boot.trn_boot import _ntff_profile_via_ctypes
        mod = types.ModuleType("antenv.axon_hooks")
        mod._hook = _ntff_profile_via_ctypes("/opt/axon/libaxon_pjrt.so")
        mod.get_axon_ntff_profile_hook = lambda: mod._hook
        mod.set_axon_ntff_profile_hook = lambda h: setattr(mod, "_hook", h)
        sys.modules["antenv.axon_hooks"] = mod
        antenv.axon_hooks = mod
    except Exception:
        pass


def kernel(emb, edge_index, W1, b1, W2, b2):
    global LAST_RESULTS
    from concourse.bass_utils import run_bass_kernel_spmd
    _ensure_ntff_hook()

    emb = np.asarray(emb, dtype=np.float32)
    W1 = np.asarray(W1, dtype=np.float32)
    W2 = np.asarray(W2, dtype=np.float32)
    b1 = np.asarray(b1, dtype=np.float32)
    b2 = np.asarray(b2, dtype=np.float32)
    assert np.abs(b1).max() == 0 and np.abs(b2).max() == 0, \
        "nonzero biases not implemented"
    col = np.asarray(edge_index[0]).astype(np.int64)
    row = np.asarray(edge_index[1]).astype(np.int64)

    n, h = emb.shape
    assert h == H
    E = col.shape[0]
    n_pad = ((n + P - 1) // P) * P
    qsz = math.ceil(n_pad / NBUCKET / ST_W) * ST_W
    n_blocks = n_pad // H

    # ---- global (row_bucket, col) sort, round-robin deal to cores ----
    qb = np.minimum(row // qsz, NBUCKET - 1).astype(np.int8)
    order = np.lexsort((col, qb))        # edge ids, bucket-0 first
    bnds = np.searchsorted(qb[order], np.arange(NBUCKET + 1))
    buckets = tuple(order[bnds[b]:bnds[b + 1]] for b in range(NBUCKET))

    percore_c = [[None] * NBUCKET for _ in range(NCORES)]
    percore_r = [[None] * NBUCKET for _ in range(NCORES)]
    nck = []
    for b in range(NBUCKET):
        ids = buckets[b]
        cnt_max = len(ids[0::NCORES]) if len(ids) else 0
        nst = math.ceil(cnt_max / ST) if cnt_max else 0
        nck.append(nst)
        tgt = nst * ST
        for ci in range(NCORES):
            sel = ids[ci::NCORES]
            cs = col[sel]
            rs = row[sel] - qsz * b
            pad = tgt - len(cs)
            fillc = cs[-1] if len(cs) else 0
            percore_c[ci][b] = np.concatenate(
                [cs, np.full(pad, fillc, dtype=cs.dtype)])
            percore_r[ci][b] = np.concatenate(
                [rs, np.zeros(pad, dtype=rs.dtype)])
    nck = tuple(nck)
    n_st = sum(nck)
    n_chunks = n_st * (ST // CH)
    n_groups = math.ceil(n_st / P)

    # merged per-chunk block structure
    b0s = np.empty((NCORES, n_chunks), dtype=np.int64)
    his = np.empty((NCORES, n_chunks), dtype=np.int64)
    for ci in range(NCORES):
        cp = np.concatenate(percore_c[ci])
        cc = cp.reshape(n_chunks, CH)
        b0s[ci] = cc[:, 0] >> 7
        his[ci] = cc[:, -1] >> 7
    b0m = b0s.min(axis=0)
    him = his.max(axis=0)
    spans = him - b0m + 1
    assert spans.max() <= MAXSPAN, f"merged span {spans.max()} > {MAXSPAN}"
    chunk_meta = tuple(zip(b0m.tolist(), spans.tolist()))

    key = (n_pad, qsz, nck, chunk_meta)
    if key not in _PROG_CACHE:
        _PROG_CACHE[key] = _build_program(n_pad, qsz, nck, chunk_meta)
    nc = _PROG_CACHE[key]

    # ---- inputs ----
    embT = np.zeros((P, n_pad), dtype=BF16)
    embT[:, :n] = emb.T.astype(BF16)
    w1t = W1.T.astype(BF16)
    w2t = W2.T.astype(BF16)
    iota = np.empty((P, MAXSPAN * ST), dtype=np.float16)
    base = np.arange(P, dtype=np.float16)[:, None]
    for j in range(MAXSPAN):
        iota[:, j * ST:(j + 1) * ST] = base + j * CH

    in_maps = []
    for ci in range(NCORES):
        cp = np.concatenate(percore_c[ci])
        rp = np.concatenate(percore_r[ci])
        crel = (cp.reshape(n_chunks, CH)
                - (b0m[:, None] << 7)).astype(np.float16).reshape(1, -1)
        assert crel.min() >= 0 and crel.max() < MAXSPAN * CH
        in_maps.append({
            "embT": embT, "w1t": w1t, "w2t": w2t,
            "ridx": _wrap_idx(rp.astype(np.int16)),
            "crel": crel, "iota": iota,
        })

    res = run_bass_kernel_spmd(nc, in_maps, core_ids=list(range(NCORES)))
    LAST_RESULTS = res

    # ---- reassemble ----
    out = np.empty(E, dtype=np.float32)
    streams = [res.results[ci]["out"].reshape(-1) for ci in range(NCORES)]
    for b in range(NBUCKET):
        ids = buckets[b]
        off = sum(nck[:b]) * ST
        for ci in range(NCORES):
            sel = ids[ci::NCORES]
            out[sel] = streams[ci][off:off + len(sel)]
    return out
